# revision 1
# baseline (speedup 1.0000x reference)
"""Trainium2 Bass kernel for DeformAxialDW.

Reference computes: out = x + convH(x) + convW(x) where convH/convW are
depthwise 1D convs (7 taps) along H/W with fractional dilation r realized
as bilinear sampling. Expanding the bilinear interpolation over integer
shifts, each conv becomes a per-channel banded (Toeplitz) conv with
2S+1 integer taps, S = floor(3*r)+1.

Per-core plan (1 batch item per NeuronCore, 8 cores):
  - layout: h on SBUF partitions, w in free dim; x split into two aligned
    112-row blocks (rows 0:112 and 112:224), one pair of tiles per channel
  - H-conv: per-channel banded Toeplitz stationary (bf16) x moving (bf16)
    matmuls into fp32 PSUM; cross-block halo handled by "edge" matmuls
    whose Toeplitz is zero except a small corner
  - W-conv: PE-transpose 112x112 blocks of x, then matmul with the
    transposed block as stationary and the per-channel W-Toeplitz as
    moving, accumulated into the SAME PSUM tile as the H-conv
  - identity (+x): fp32 add on VectorE while copying PSUM->SBUF
  - fp32->bf16 casts on GpSimd, PSUM->SBUF transpose copies on ScalarE
"""

import sys

import numpy as np

sys.path.insert(0, "/opt/trn_rl_repo")

import ml_dtypes

BF16 = ml_dtypes.bfloat16

C, H, W = 128, 224, 224
B = 8
HS = 112  # row-block / h_out / w_in block size

_CACHE = {}


def _tap_coeffs(w_taps: np.ndarray, r_val: float, S: int) -> np.ndarray:
    """Expand 7 fractional-dilation taps into 2S+1 integer-shift coeffs."""
    Cn, K = w_taps.shape
    P = K // 2
    alpha = np.zeros((Cn, 2 * S + 1), dtype=np.float64)
    for i in range(K):
        k_pos = i - P
        delta = np.float32(k_pos) * np.float32(r_val)
        d0 = int(np.floor(delta))
        frac = float(np.float32(delta) - np.float32(d0))
        alpha[:, d0 + S] += (1.0 - frac) * w_taps[:, i].astype(np.float64)
        alpha[:, d0 + 1 + S] += frac * w_taps[:, i].astype(np.float64)
    return alpha


def _banded(alpha: np.ndarray, rows: int, cols: int, diag_off: int, S: int):
    """M[i, c, jj] = alpha[c, (i - jj + diag_off)] where |i-jj+diag_off|<=S."""
    Cn = alpha.shape[0]
    out = np.zeros((rows, Cn, cols), dtype=np.float64)
    i = np.arange(rows)[:, None]
    jj = np.arange(cols)[None, :]
    d = i - jj + diag_off
    mask = np.abs(d) <= S
    ii, jjj = np.nonzero(mask)
    out[ii, :, jjj] = alpha[:, d[ii, jjj] + S].T
    return out


def _build_nc(S: int, repeat: int = 1):
    import concourse.mybir as mybir
    from concourse import bacc
    from concourse.tile import TileContext

    f32 = mybir.dt.float32
    bf16 = mybir.dt.bfloat16

    nc = bacc.Bacc("TRN2", target_bir_lowering=False, debug=False)
    x_p = nc.declare_dram_parameter("x", [C, H, W], f32, isOutput=False)
    gh_p = nc.declare_dram_parameter("gh", [HS, C, HS], bf16, isOutput=False)
    gw_p = nc.declare_dram_parameter("gw", [HS, C, HS + 3 * S], bf16, isOutput=False)
    # corner (cross-block halo) stationaries for the H-conv edge matmuls:
    # ce0: h_in block1 rows [112,144) -> h_out [96,112);
    # ce1: h_in block0 rows [64,112) -> h_out [112,128)
    ce0_p = nc.declare_dram_parameter("ce0", [32, C, 16], bf16, isOutput=False)
    ce1_p = nc.declare_dram_parameter("ce1", [48, C, 16], bf16, isOutput=False)
    id_p = nc.declare_dram_parameter("ident", [HS, HS], bf16, isOutput=False)
    out_p = nc.declare_dram_parameter("out", [C, H, W], f32, isOutput=True)

    G = 8  # channels per DMA group
    with TileContext(nc) as tc:
        with tc.tile_pool(name="const", bufs=1) as constp, \
             tc.tile_pool(name="xf", bufs=3) as xfp, \
             tc.tile_pool(name="xb", bufs=3) as xbp, \
             tc.tile_pool(name="gt", bufs=3) as gtp, \
             tc.tile_pool(name="xt", bufs=6) as xtp, \
             tc.tile_pool(name="outs", bufs=3) as outp, \
             tc.tile_pool(name="pp", bufs=4, space="PSUM") as ppp, \
             tc.tile_pool(name="po", bufs=4, space="PSUM") as pop:
            ident = constp.tile([HS, HS], bf16)
            nc.sync.dma_start(out=ident[:, :], in_=id_p[:, :])
            for _rep in range(repeat):
              for c0 in range(0, C, G):
                  ghg = gtp.tile([HS, G, HS], bf16, tag="gh")
                  gwg = gtp.tile([HS, G, HS + 3 * S], bf16, tag="gw")
                  nc.sync.dma_start(out=ghg[:, :, :], in_=gh_p[:, c0:c0 + G, :])
                  nc.sync.dma_start(out=gwg[:, :, :], in_=gw_p[:, c0:c0 + G, :])
                  ce0g = gtp.tile([32, G, 16], bf16, tag="ce0")
                  ce1g = gtp.tile([HS, G, 16], bf16, tag="ce1")
                  nc.sync.dma_start(out=ce0g[:, :, :], in_=ce0_p[:, c0:c0 + G, :])
                  # ce1 occupies partitions [64,112) so the matmul reading
                  # xb[0][64:112] sees matching base partitions
                  nc.sync.dma_start(out=ce1g[64:HS, :, :], in_=ce1_p[:, c0:c0 + G, :])
                  xf = []
                  xb = []
                  for t in (0, 1):
                      xf_t = xfp.tile([HS, G, W], f32, tag=f"xf{t}")
                      nc.sync.dma_start(
                          out=xf_t[:, :, :],
                          in_=x_p[c0:c0 + G, t * HS:(t + 1) * HS, :].rearrange(
                              "c h w -> h c w"
                          ),
                      )
                      xb_t = xbp.tile([HS, G, W], bf16, tag=f"xb{t}")
                      nc.gpsimd.tensor_copy(out=xb_t[:, :, :], in_=xf_t[:, :, :])
                      xf.append(xf_t)
                      xb.append(xb_t)
                  og0 = outp.tile([HS, G, W], f32, tag="ot0")
                  og1 = outp.tile([HS, G, W], f32, tag="ot1")
                  og = [og0, og1]
                  for cl in range(G):
                      # transpose x blocks: xts[q][:, t, :] = x[tblock_t, wchunk_q].T
                      xts = []
                      for q in (0, 1):
                          xt_t = xtp.tile([HS, 2, HS], bf16, tag=f"xt{q}")
                          pp = ppp.tile([HS, 2, HS], bf16)
                          for t in (0, 1):
                              nc.tensor.matmul(
                                  out=pp[:, t, :],
                                  lhsT=xb[t][0:HS, cl, q * HS:(q + 1) * HS],
                                  rhs=ident[:, :],
                                  is_transpose=True,
                                  skip_group_check=True,
                              )
                          nc.scalar.copy(out=xt_t[:, :, :], in_=pp[:, :, :])
                          xts.append(xt_t)
                      for t in (0, 1):
                          po = pop.tile([HS, W], f32)
                          # H-conv: main (same-block) + edge (other block)
                          nc.tensor.matmul(
                              out=po[:, :],
                              lhsT=ghg[0:HS, cl, :],
                              rhs=xb[t][0:HS, cl, :],
                              start=True, stop=False,
                          )
                          if t == 0:
                              nc.tensor.matmul(
                                  out=po[96:HS, :],
                                  lhsT=ce0g[0:32, cl, :],
                                  rhs=xb[1][0:32, cl, :],
                                  start=False, stop=False,
                                  tile_position=(0, 96),
                              )
                          else:
                              nc.tensor.matmul(
                                  out=po[0:16, :],
                                  lhsT=ce1g[64:HS, cl, :],
                                  rhs=xb[0][64:HS, cl, :],
                                  start=False, stop=False,
                              )
                          # W-conv: two w_in chunks
                          nc.tensor.matmul(
                              out=po[0:HS, 0:HS + S],
                              lhsT=xts[0][0:HS, t, :],
                              rhs=gwg[0:HS, cl, 2 * S:3 * S + HS],
                              start=False, stop=False,
                          )
                          nc.tensor.matmul(
                              out=po[0:HS, HS - S:W],
                              lhsT=xts[1][0:HS, t, :],
                              rhs=gwg[0:HS, cl, S:2 * S + HS],
                              start=False, stop=True,
                          )
                          nc.vector.tensor_add(
                              out=og[t][:, cl, :], in0=xf[t][0:HS, cl, :], in1=po[:, :]
                          )
                  for t in (0, 1):
                      # stores ride the second HWDGE ring (ACT) so they don't
                      # block the sync-engine load queue
                      nc.scalar.dma_start(
                          out=out_p[c0:c0 + G, t * HS:(t + 1) * HS, :].rearrange(
                              "c h w -> h c w"
                          ),
                          in_=og[t][:, :, :],
                      )
    nc.compile()
    return nc


def _prepare_consts(weight_h, weight_w, r):
    r_val = float(max(np.float32(r), np.float32(1.0)))
    S = int(np.floor(3.0 * r_val)) + 1
    assert S <= 16, f"dilation r={r_val} too large for this kernel (S={S})"
    wh = np.asarray(weight_h)[:, 0, :, 0].astype(np.float64)
    ww = np.asarray(weight_w)[:, 0, 0, :].astype(np.float64)
    ah = _tap_coeffs(wh, r_val, S)
    aw = _tap_coeffs(ww, r_val, S)
    gh = _banded(ah, HS, HS, 0, S).astype(BF16)
    gw = _banded(aw, HS, HS + 3 * S, 2 * S, S).astype(BF16)
    # corner stationaries: ce0[i,c,j] = ah[(112+i)-(96+j)], i in [0,32), j in [0,16)
    # ce1[i,c,j] = ah[(64+i)-(112+j)], i in [0,48), j in [0,16)
    ce0 = _banded(ah, 32, 16, 16, S).astype(BF16)
    ce1 = _banded(ah, 48, 16, -48, S).astype(BF16)
    ident = np.eye(HS, dtype=BF16)
    return S, gh, gw, ce0, ce1, ident


def kernel(x, weight_h, weight_w, r):
    from concourse.bass_utils import run_bass_kernel_spmd

    x = np.asarray(x, dtype=np.float32)
    assert x.shape == (B, C, H, W), x.shape
    S, gh, gw, ce0, ce1, ident = _prepare_consts(weight_h, weight_w, r)

    if S not in _CACHE:
        _CACHE[S] = _build_nc(S)
    nc = _CACHE[S]

    in_maps = [
        {"x": x[b], "gh": gh, "gw": gw, "ce0": ce0, "ce1": ce1, "ident": ident}
        for b in range(B)
    ]
    res = run_bass_kernel_spmd(nc, in_maps, core_ids=list(range(B)))
    out = np.stack([res.results[b]["out"] for b in range(B)], axis=0)
    return out



# revision 7
# speedup vs baseline: 1.5189x; 1.5189x over previous
"""Trainium2 Bass kernel for DeformAxialDW.

Reference computes out = x + convH(x) + convW(x): depthwise 7-tap 1D convs
along H and W with fractional dilation r (bilinear sampling), which expand
into per-channel banded (Toeplitz) convs with 2S+1 integer taps,
S = floor(3*r)+1.

Layout/precision plan (per core = one batch item, 8 cores):
  - x is packed on the HOST to bf16 [2, 112+S, C, W]: two h-blocks with 2S
    rows of overlap (rows [0,112+S) and [112-S,224)).  The overlap lets each
    output block's H-conv be a single [112+S -> 112] banded matmul with NO
    edge/corner matmuls.
  - The identity (+x) is folded into the Toeplitz masters (+0.5 on the
    center tap of both the H and W masters), so out = Hconv' + Wconv'
    accumulates entirely in PSUM; no separate add pass.
  - One H master MH [112+2S, C, 112] is shared by both blocks via partition
    slices; one W master MW [112, C, 112+2S] is shared by both w-chunks via
    free-dim slices.
  - W-conv needs x transposed: 4 PE transposes per channel (bf16, via
    permutation matmul) -> PSUM -> one DVE copy to SBUF; the transposed
    chunks are the matmul *stationary* (stationary load is cheap), with the
    W master as the moving operand.
  - PSUM po tiles hold 2 channels padded to 256 f32 each (1 bank, no
    matmul bank crossing); f32->bf16 output copies run mostly on GpSimd
    (best cost/elem), every 4th on DVE.
  - Output bf16 [2, 112, C, W], unpacked + upcast on the host.
"""

import sys

import numpy as np

sys.path.insert(0, "/opt/trn_rl_repo")

import ml_dtypes

BF16 = ml_dtypes.bfloat16

C, H, W = 128, 224, 224
B = 8
HO = 112  # output rows per h-block

_CACHE = {}


def _tap_coeffs(w_taps: np.ndarray, r_val: float, S: int) -> np.ndarray:
    """Expand 7 fractional-dilation taps into 2S+1 integer-shift coeffs."""
    Cn, K = w_taps.shape
    P = K // 2
    alpha = np.zeros((Cn, 2 * S + 1), dtype=np.float64)
    for i in range(K):
        k_pos = i - P
        delta = np.float32(k_pos) * np.float32(r_val)
        d0 = int(np.floor(delta))
        frac = float(np.float32(delta) - np.float32(d0))
        alpha[:, d0 + S] += (1.0 - frac) * w_taps[:, i].astype(np.float64)
        alpha[:, d0 + 1 + S] += frac * w_taps[:, i].astype(np.float64)
    return alpha


def _banded(alpha: np.ndarray, rows: int, cols: int, diag_off: int, S: int):
    """M[i, c, jj] = alpha[c, (i - jj + diag_off) + S] where |i-jj+diag_off|<=S."""
    Cn = alpha.shape[0]
    out = np.zeros((rows, Cn, cols), dtype=np.float64)
    i = np.arange(rows)[:, None]
    jj = np.arange(cols)[None, :]
    d = i - jj + diag_off
    mask = np.abs(d) <= S
    ii, jjj = np.nonzero(mask)
    out[ii, :, jjj] = alpha[:, d[ii, jjj] + S].T
    return out


def _build_nc(S: int):
    import concourse.mybir as mybir
    from concourse import bacc
    from concourse.tile import TileContext

    f32 = mybir.dt.float32
    bf16 = mybir.dt.bfloat16

    HT = HO + 2 * S    # x tile rows per block incl. S zero-pad rows (124)
    MR = HO + 2 * S    # H master rows / W master cols (112+2S)
    WS = HO + S        # W-conv moving width per chunk (112+S)

    nc = bacc.Bacc("TRN2", target_bir_lowering=False, debug=False)
    x_p = nc.declare_dram_parameter("x", [2, HT, C, W], bf16, isOutput=False)
    mh_p = nc.declare_dram_parameter("mh", [MR, C, HO], bf16, isOutput=False)
    mw_p = nc.declare_dram_parameter("mw", [HO, C, MR], bf16, isOutput=False)
    id_p = nc.declare_dram_parameter("ident", [HT, HT], bf16, isOutput=False)
    out_p = nc.declare_dram_parameter("out", [2, HO, C, W], bf16, isOutput=True)

    G = 16  # channels per DMA / store group
    with TileContext(nc) as tc:
        with tc.tile_pool(name="const", bufs=1) as constp, \
             tc.tile_pool(name="xg", bufs=2) as xgp, \
             tc.tile_pool(name="xt", bufs=3) as xtp, \
             tc.tile_pool(name="og", bufs=2) as ogp, \
             tc.tile_pool(name="pp", bufs=2, space="PSUM") as ppp, \
             tc.tile_pool(name="po", bufs=3, space="PSUM") as pop:
            ident = constp.tile([HT, HT], bf16)
            nc.sync.dma_start(out=ident[:, :], in_=id_p[:, :])
            mh = constp.tile([MR, C, HO], bf16, tag="mh")
            mw = constp.tile([HO, C, MR], bf16, tag="mw")
            ncopy = 0
            for g in range(C // G):
                c0 = g * G
                # chunked master loads so the first channels start early
                nc.sync.dma_start(out=mh[:, c0:c0 + G, :], in_=mh_p[:, c0:c0 + G, :])
                nc.sync.dma_start(out=mw[:, c0:c0 + G, :], in_=mw_p[:, c0:c0 + G, :])
                xg = []
                for t in (0, 1):
                    xg_t = xgp.tile([HT, G, W], bf16, tag=f"xg{t}")
                    nc.sync.dma_start(out=xg_t[:, :, :], in_=x_p[t, :, c0:c0 + G, :])
                    xg.append(xg_t)
                og0 = ogp.tile([HO, G, W], bf16, tag="og0")
                og1 = ogp.tile([HO, G, W], bf16, tag="og1")
                og = [og0, og1]
                po = [None, None]
                for cl in range(G):
                    c = c0 + cl
                    # transpose both w-chunks of both blocks: pp[:, 2t+q, :]
                    pp = ppp.tile([HO, 4, HT], bf16)
                    for t in (0, 1):
                        for q in (0, 1):
                            nc.tensor.matmul(
                                out=pp[:, 2 * t + q, :],
                                lhsT=xg[t][0:HT, cl, q * HO:(q + 1) * HO],
                                rhs=ident[:, :],
                                is_transpose=True,
                                skip_group_check=True,
                            )
                    xt = xtp.tile([HO, 4, HO], bf16)
                    nc.vector.tensor_copy(out=xt[:, :, :], in_=pp[:, :, S:S + HO])
                    if cl % 2 == 0:
                        po_t0 = pop.tile([HO, 2, 256], f32, tag="po0")
                        po_t1 = pop.tile([HO, 2, 256], f32, tag="po1")
                        po = [po_t0, po_t1]
                    sl = cl % 2
                    for t in (0, 1):
                        # H-conv (+0.5 identity): banded [HT->HO] stationary,
                        # x block moving (zero pad rows contribute nothing)
                        nc.tensor.matmul(
                            out=po[t][:, sl, 0:W],
                            lhsT=mh[0:HT, c, :],
                            rhs=xg[t][0:HT, cl, :],
                            start=True, stop=False,
                        )
                        # W-conv (+0.5 identity): transposed-x stationary,
                        # W master moving; two w_in chunks
                        nc.tensor.matmul(
                            out=po[t][:, sl, 0:WS],
                            lhsT=xt[0:HO, 2 * t, :],
                            rhs=mw[0:HO, c, S:S + WS],
                            start=False, stop=False,
                        )
                        nc.tensor.matmul(
                            out=po[t][:, sl, HO - S:W],
                            lhsT=xt[0:HO, 2 * t + 1, :],
                            rhs=mw[0:HO, c, 0:WS],
                            start=False, stop=True,
                        )
                    if cl % 2 == 1:
                        for t in (0, 1):
                            src = po[t][:, :, 0:W]
                            dst = og[t][:, cl - 1:cl + 1, :]
                            # GPSIMD cannot read PSUM; balance DVE vs ACT
                            if ncopy % 7 == 6:
                                nc.vector.tensor_copy(out=dst, in_=src)
                            else:
                                nc.scalar.copy(out=dst, in_=src)
                            ncopy += 1
                for t in (0, 1):
                    # stores ride the ACT HWDGE ring so they don't block the
                    # sync-engine load queue
                    nc.scalar.dma_start(
                        out=out_p[t, :, c0:c0 + G, :], in_=og[t][:, :, :]
                    )
    nc.compile()
    return nc


def _prepare_consts(weight_h, weight_w, r):
    r_val = float(max(np.float32(r), np.float32(1.0)))
    S = int(np.floor(3.0 * r_val)) + 1
    assert S <= 16, f"dilation r={r_val} too large for this kernel (S={S})"
    wh = np.asarray(weight_h)[:, 0, :, 0].astype(np.float64)
    ww = np.asarray(weight_w)[:, 0, 0, :].astype(np.float64)
    ah = _tap_coeffs(wh, r_val, S)
    aw = _tap_coeffs(ww, r_val, S)
    # fold the identity: each conv contributes x/2 via its center tap
    ah[:, S] += 0.5
    aw[:, S] += 0.5
    mh = _banded(ah, HO + 2 * S, HO, -S, S).astype(BF16)
    mw = _banded(aw, HO, HO + 2 * S, S, S).astype(BF16)
    ident = np.eye(HO + 2 * S, dtype=BF16)
    return S, mh, mw, ident


def kernel(x, weight_h, weight_w, r):
    from concourse.bass_utils import run_bass_kernel_spmd

    x = np.asarray(x, dtype=np.float32)
    assert x.shape == (B, C, H, W), x.shape
    S, mh, mw, ident = _prepare_consts(weight_h, weight_w, r)
    HT = HO + 2 * S

    if S not in _CACHE:
        _CACHE[S] = _build_nc(S)
    nc = _CACHE[S]

    xb = x.astype(BF16)
    in_maps = []
    for b in range(B):
        pk = np.zeros((2, HT, C, W), dtype=BF16)
        pk[0, S:HT] = xb[b, :, 0:HO + S].transpose(1, 0, 2)
        pk[1, 0:HO + S] = xb[b, :, HO - S:H].transpose(1, 0, 2)
        in_maps.append({"x": pk, "mh": mh, "mw": mw, "ident": ident})

    res = run_bass_kernel_spmd(nc, in_maps, core_ids=list(range(B)))
    out = np.empty((B, C, H, W), dtype=np.float32)
    for b in range(B):
        o = np.asarray(res.results[b]["out"])  # (2, HO, C, W) bf16
        out[b, :, 0:HO] = o[0].transpose(1, 0, 2)
        out[b, :, HO:H] = o[1].transpose(1, 0, 2)
    return out
